# revision 9
# baseline (speedup 1.0000x reference)
"""Trainium2 Bass kernel for nn_BaselineModel_35175782154746 (dense transformer
block with SiLU attention + relative-position bias).

Sharding: 8 NeuronCores = 4 batches x 2 head-groups (8 heads each).
Each core computes, for its (batch b, head-group g):
    U, Q, K, V projections (columns g*1024:(g+1)*1024 of Wu/Wq/Wk/Wv),
    SiLU attention with rel-pos bias for its 8 heads,
    gated = out * U (written in place over U), partial = gated @ Wf2 rows.
Host reduces: out[b] = partial[2b] + partial[2b+1] + bf2.

v2 schedule (vs the 362.7us baseline):
  - causal fine-trim: score/AV blocks narrowed to the causal region at
    128-granularity; two trimmed blocks pack into one PSUM bank (the
    second with start=False, landing on the bank's pending-zero range).
  - projection halves end with the last 4 k-tiles h-grouped so PSUM
    banks free progressively instead of all at once behind a serial
    silu drain.
  - scores(ih=0) run between K and V, interleaved with V's sb-blocks;
    AV(ih=0) interleaves with V's second column half.  V bias is a DVE
    add (no ones-matmul).  gated overwrites UT in place (saves 2MB).
  - ih=1 loop is software-pipelined (AV of head h-1 after scores of h),
    f2 part-A blocks spread through it, f2 copies on the Scalar engine,
    per-(head,ih) merged silu, output staged/stored as bf16.
  - DMA issue spread over sync/scalar/vector/gpsimd queues.
"""

import sys
import os

for _p in ("/root/.axon_site/_ro/trn_rl_repo", "/opt/trn_rl_repo"):
    if os.path.isdir(_p) and _p not in sys.path:
        sys.path.append(_p)

import numpy as np

import concourse.bass as bass
import concourse.mybir as mybir
import concourse.tile as tile
from concourse import bacc
from concourse.bass_utils import run_bass_kernel_spmd

B, S, H, NH, MAXLEN = 4, 1024, 2048, 16, 1024
HD = H // NH            # 128
NHL = 8                 # heads per core (local)
HGRP = 2                # head groups
NCORES = 8
KT16 = H // 128         # 16 k-tiles for the H contraction
SCALE = float(HD) ** -0.5

f32 = mybir.dt.float32
bf16 = mybir.dt.bfloat16
SILU = mybir.ActivationFunctionType.Silu
MULT = mybir.AluOpType.mult
ADD = mybir.AluOpType.add

TRACE = False
LAST_EXEC_NS = None
_CACHE = {}


def _blocks(causal, ih):
    """Score blocks for query half ih, in PSUM-group order.

    Each entry is (jb, qoff, width, packed): qoff the absolute query
    offset, width the trimmed moving size; packed means the block shares
    the previous block's PSUM bank, written at the next free column.
    """
    if not causal:
        return [(jb, ih * 512, 512, False) for jb in range(8)]
    if ih == 0:
        # widths 512,384,256,128 -> bank groups [jb0], [jb1,jb3], [jb2]
        return [(0, 0, 512, False), (1, 128, 384, False), (3, 384, 128, True),
                (2, 256, 256, False)]
    # ih == 1: jb0..4 full, then [jb5,jb7] packed, jb6 alone
    out = [(jb, 512, 512, False) for jb in range(5)]
    out += [(5, 640, 384, False), (7, 896, 128, True), (6, 768, 256, False)]
    return out


def _att_layout(causal, ih):
    """att tile column ranges: dict jb -> (col_start, width, qoff)."""
    lay = {}
    c = 0
    for jb, qoff, w, packed in _blocks(causal, ih):
        lay[jb] = (c, w, qoff)
        c += w
    return lay, c


def _build(causal: bool):
    nc = bacc.Bacc("TRN2", target_bir_lowering=False, debug=False,
                   num_devices=NCORES)

    def din(name, shape, dt=f32):
        return nc.dram_tensor(name, shape, dt, kind="ExternalInput").ap()

    qT = din("qT", [H, S], bf16)
    kT = din("kT", [H, S], bf16)
    vT = din("vT", [H, S], bf16)
    wq = din("wq", [H, NHL * HD], bf16)
    wk = din("wk", [H, NHL * HD], bf16)
    wv = din("wv", [H, NHL * HD], bf16)
    wu = din("wu", [H, NHL * HD], bf16)
    wf2 = din("wf2", [NHL * HD, H], bf16)
    bq = din("bq", [128, NHL])
    bk = din("bk", [128, NHL])
    bu = din("bu", [128, NHL])
    bvb = din("bvb", [128, NHL * HD], bf16)
    ATW = 1024 if causal else 2047
    atab = din("atab", [NHL, 128, ATW], bf16)
    if not causal:
        maskf = din("maskf", [128, NHL, S], bf16)
    out = nc.dram_tensor("out", [S, H], bf16, kind="ExternalOutput").ap()

    wf2r = wf2.rearrange("(cb p) n -> p cb n", p=128)

    lay0, W0 = _att_layout(causal, 0)
    lay1, W1 = _att_layout(causal, 1)

    with tile.TileContext(nc) as tc:
        with (
            tc.tile_pool(name="const", bufs=1) as constp,
            tc.tile_pool(name="projout", bufs=1) as projp,
        ):
            bq_t = constp.tile([128, NHL], f32, tag="bq")
            bk_t = constp.tile([128, NHL], f32, tag="bk")
            bu_t = constp.tile([128, NHL], f32, tag="bu")
            bvb_t = constp.tile([128, NHL * HD], bf16, tag="bvb")

            UT = projp.tile([128, NHL, S], bf16, tag="UT")
            QT = projp.tile([128, NHL, S], bf16, tag="QT")
            KTt = projp.tile([128, NHL, S], bf16, tag="KT")
            V = projp.tile([128, NHL, S], bf16, tag="V")
            at_tiles = [projp.tile([128, ATW], bf16, tag=f"atab{h}",
                                   name=f"atab{h}")
                        for h in range(NHL)]
            if not causal:
                mask_t = projp.tile([128, NHL, S], bf16, tag="mask")

            # ---- emit helpers (pool passed per phase) ----
            def emit_scores(h, ih, att_t, lay, pool, tagger):
                """score matmuls + STT(scale, +rel-bias) for (head, half)."""
                blocks = _blocks(causal, ih)
                at = at_tiles[h]
                scp = None
                gi = 0
                off = 0
                stts = []
                for bi, (jb, qoff, w, packed) in enumerate(blocks):
                    if not packed:
                        scp = pool.tile([128, 512], f32, tag=tagger(gi),
                                        name=f"sc{h}_{ih}_{jb}")
                        gi += 1
                        off = 0
                    grp_end = (bi + 1 >= len(blocks)
                               or not blocks[bi + 1][3])
                    nc.tensor.matmul(
                        scp[:, off:off + w],
                        lhsT=KTt[:, h, jb * 128:(jb + 1) * 128],
                        rhs=QT[:, h, qoff:qoff + w],
                        start=(not packed), stop=grp_end)
                    stts.append((scp, off, jb, qoff, w))
                    off += w
                for scp, off, jb, qoff, w in stts:
                    c0 = lay[jb][0]
                    d0 = qoff - jb * 128 + (0 if causal else MAXLEN - 1)
                    nc.vector.scalar_tensor_tensor(
                        att_t[:, c0:c0 + w], scp[:, off:off + w],
                        SCALE, at[:, d0:d0 + w], op0=MULT, op1=ADD)

            def emit_post(h, att_t, lay):
                """merged silu (+ mask for the dense variant)."""
                nc.scalar.activation(att_t[:], att_t[:], SILU)
                if not causal:
                    for jb in sorted(lay):
                        c0, w, qoff = lay[jb]
                        nc.vector.tensor_mul(
                            att_t[:, c0:c0 + w], att_t[:, c0:c0 + w],
                            mask_t[:, jb, qoff:qoff + w])

            def emit_av(h, ih, att_t, lay, avp):
                """AV accumulation + gated-mul written in place over UT."""
                base = ih * 512
                njb = len(lay)
                done = 0
                for jb in sorted(lay):
                    c0, w, qoff = lay[jb]
                    o = qoff - base
                    done += 1
                    nc.tensor.matmul(
                        avp[:, o:o + w],
                        lhsT=V[:, jb, h * HD:(h + 1) * HD],
                        rhs=att_t[:, c0:c0 + w],
                        start=(jb == 0), stop=(done == njb))
                nc.vector.tensor_mul(
                    UT[:, h, base:base + 512], avp[:],
                    UT[:, h, base:base + 512])

            with tc.tile_pool(name="inres", bufs=1) as inres:
                qres = inres.tile([128, KT16, S], bf16, tag="qres")
                kres = inres.tile([128, KT16, S], bf16, tag="kres")
                vres = inres.tile([128, KT16, S], bf16, tag="qres",
                                  name="vres")
                wvres = inres.tile([128, KT16, NHL * HD], bf16, tag="wv",
                                   name="wvres")
                # input DMAs.  sync: qres then vres then wvres (order
                # matters: vres waits for Q to release the qres slot, and
                # wvres transfers land during K).  scalar: biases, kres,
                # atab (+mask).  vector/gpsimd: projection weight stream.
                for k in range(KT16):
                    nc.sync.dma_start(qres[:, k, :],
                                      qT[k * 128:(k + 1) * 128, :])
                nc.scalar.dma_start(bu_t[:], bu[:])
                nc.scalar.dma_start(bq_t[:], bq[:])
                nc.scalar.dma_start(bk_t[:], bk[:])
                nc.scalar.dma_start(bvb_t[:], bvb[:])
                for h in range(NHL):
                    nc.scalar.dma_start(at_tiles[h][:], atab[h])
                if not causal:
                    nc.scalar.dma_start(mask_t[:], maskf[:])
                for k in range(KT16):
                    nc.scalar.dma_start(kres[:, k, :],
                                        kT[k * 128:(k + 1) * 128, :])
                for k in range(KT16):
                    nc.sync.dma_start(vres[:, k, :],
                                      vT[k * 128:(k + 1) * 128, :])
                for k in range(KT16):
                    nc.sync.dma_start(wvres[:, k, :],
                                      wv[k * 128:(k + 1) * 128, :])

                # ---------------- projections U, Q, K ----------------
                with (
                    tc.tile_pool(name="win", bufs=7) as winp,
                    tc.tile_pool(name="pps", bufs=1, space="PSUM") as ppsum,
                ):
                    KSPLIT = 12   # k-outer prefix; tail h-grouped

                    for wdram, xres, btile, outtile, st_i in (
                        (wu, qres, bu_t, UT, 0),
                        (wq, qres, bq_t, QT, 1),
                        (wk, kres, bk_t, KTt, 2),
                    ):
                        for ih in range(2):
                            wts = {}
                            for k in range(KT16):
                                wt = winp.tile([128, NHL * HD], bf16,
                                               tag="win")
                                nc.gpsimd.dma_start(
                                    wt[:], wdram[k * 128:(k + 1) * 128, :])
                                wts[k] = wt
                            ps = [ppsum.tile([128, 512], f32, tag=f"pp{h}",
                                             name=f"pp{st_i}_{ih}_{h}")
                                  for h in range(NHL)]
                            sl = slice(ih * 512, (ih + 1) * 512)
                            for k in range(KSPLIT):
                                for h in range(NHL):
                                    nc.tensor.matmul(
                                        ps[h][:],
                                        lhsT=wts[k][:, h * HD:(h + 1) * HD],
                                        rhs=xres[:, k, sl],
                                        start=(k == 0), stop=False)
                            for h in range(NHL):
                                for k in range(KSPLIT, KT16):
                                    nc.tensor.matmul(
                                        ps[h][:],
                                        lhsT=wts[k][:, h * HD:(h + 1) * HD],
                                        rhs=xres[:, k, sl],
                                        start=False, stop=(k == KT16 - 1))
                                nc.scalar.activation(
                                    outtile[:, h, sl], ps[h][:], SILU,
                                    bias=btile[:, h:h + 1])

                # ------------- scores(ih=0) + V + AV(ih=0) -------------
                with (
                    tc.tile_pool(name="att0p", bufs=1) as att0p,
                    tc.tile_pool(name="psA", bufs=1, space="PSUM") as psA,
                ):
                    att0 = [att0p.tile([128, W0], bf16, tag=f"att0_{h}",
                                       name=f"att0_{h}")
                            for h in range(NHL)]

                    def a_tag(i):
                        return f"s{i % 3}_{(i // 3) % 2}"

                    def emit_v_block(ch, sb):
                        csl = slice(ch * 512, (ch + 1) * 512)
                        pv = psA.tile([128, 512], f32, tag=f"v{sb % 2}",
                                      name=f"v{ch}_{sb}")
                        for k in range(KT16):
                            nc.tensor.matmul(
                                pv[:],
                                lhsT=vres[:, k, sb * 128:(sb + 1) * 128],
                                rhs=wvres[:, k, csl],
                                start=(k == 0), stop=(k == KT16 - 1))
                        nc.vector.tensor_add(V[:, sb, csl], pv[:],
                                             bvb_t[:, csl])
                        nc.scalar.activation(V[:, sb, csl],
                                             V[:, sb, csl], SILU)

                    # loop1: scores-ih0 head i interleaved with V ch0
                    for i in range(NHL):
                        emit_scores(i, 0, att0[i], lay0, psA, a_tag)
                        emit_post(i, att0[i], lay0)
                        emit_v_block(0, i)
                    # loop2: V ch1 + AV-ih0 heads 0-3
                    for i in range(NHL):
                        emit_v_block(1, i)
                        if i % 2 == 1:
                            h = i // 2
                            avp = psA.tile([128, 512], f32,
                                           tag=f"s{h % 3}_0",
                                           name=f"av0_{h}")
                            emit_av(h, 0, att0[h], lay0, avp)
                    # loop3: AV-ih0 heads 4-7
                    for h in range(4, NHL):
                        avp = psA.tile([128, 512], f32, tag=f"s{h % 3}_1",
                                       name=f"av0_{h}")
                        emit_av(h, 0, att0[h], lay0, avp)

            # ---------------- ih=1 attention + f2 ----------------
            with (
                tc.tile_pool(name="att1p", bufs=2) as att1p,
                tc.tile_pool(name="psB", bufs=1, space="PSUM") as psB,
                tc.tile_pool(name="w2p", bufs=8) as w2p,
                tc.tile_pool(name="stgp", bufs=4) as stgp,
            ):
                w2a = []
                for n in range(4):
                    t = w2p.tile([128, NHL, 512], bf16, tag="w2",
                                 name=f"w2a{n}")
                    nc.sync.dma_start(t[:], wf2r[:, :, n * 512:(n + 1) * 512])
                    w2a.append(t)

                def b_tag(i):
                    if i < 4:
                        return f"A{i}"
                    if i < 6:
                        return f"B{i - 4}"
                    return "C"

                def emit_f2_block(w2t, n, sb, which):
                    ps = psB.tile([128, 512], f32, tag=f"A{which}",
                                  name=f"f2_{n}_{sb}")
                    for cb in range(NHL):
                        nc.tensor.matmul(
                            ps[:],
                            lhsT=UT[:, cb, sb * 128:(sb + 1) * 128],
                            rhs=w2t[:, cb, :],
                            start=(cb == 0), stop=(cb == NHL - 1))
                    st = stgp.tile([128, 512], bf16, tag="st",
                                   name=f"st{n}_{sb}")
                    nc.scalar.copy(st[:], ps[:])
                    nc.sync.dma_start(
                        out[sb * 128:(sb + 1) * 128,
                            n * 512:(n + 1) * 512], st[:])

                fa = [(n, sb) for n in range(4) for sb in range(4)]
                f2_sched = [3, 3, 2, 2, 2, 2, 1, 1]
                fpos = 0
                w2b = []
                att1_tiles = {}
                for i in range(NHL):
                    att1_tiles[i] = att1p.tile([128, W1], bf16, tag="att1",
                                               name=f"att1_{i}")
                    emit_scores(i, 1, att1_tiles[i], lay1, psB, b_tag)
                    for n, sb in fa[fpos:fpos + f2_sched[i]]:
                        emit_f2_block(w2a[n], n, sb, fpos % 2)
                        fpos += 1
                    emit_post(i, att1_tiles[i], lay1)
                    if i > 0:
                        avp = psB.tile([128, 512], f32, tag="D",
                                       name=f"av1_{i - 1}")
                        emit_av(i - 1, 1, att1_tiles.pop(i - 1), lay1, avp)
                    if i % 2 == 1:
                        t = w2p.tile([128, NHL, 512], bf16, tag="w2",
                                     name=f"w2b{i // 2}")
                        nc.gpsimd.dma_start(
                            t[:], wf2r[:, :, (i // 2) * 512:
                                       (i // 2 + 1) * 512])
                        w2b.append(t)
                avp = psB.tile([128, 512], f32, tag="D", name="av1_7")
                emit_av(7, 1, att1_tiles.pop(7), lay1, avp)

                fb = [(n, sb) for n in range(4) for sb in range(4, 8)]
                for idx, (n, sb) in enumerate(fb):
                    emit_f2_block(w2b[n], n, sb, idx % 4)

    nc.compile()
    return nc


def _host_shards(query, key, value, attn_mask, Wq, bq, Wk, bk, Wv, bv,
                 Wu, bu, Wf2, rel_table, causal):
    """Build the per-core input maps."""
    import ml_dtypes
    npdt = np.dtype(ml_dtypes.bfloat16)
    in_maps = []
    gdata = []
    for g in range(HGRP):
        c0, c1 = g * NHL * HD, (g + 1) * NHL * HD
        wq_c = np.ascontiguousarray(Wq[:, c0:c1]).astype(npdt)
        wk_c = np.ascontiguousarray(Wk[:, c0:c1]).astype(npdt)
        wv_c = np.ascontiguousarray(Wv[:, c0:c1]).astype(npdt)
        wu_c = np.ascontiguousarray(Wu[:, c0:c1]).astype(npdt)
        wf2_c = np.ascontiguousarray(Wf2[c0:c1, :]).astype(npdt)
        bq_c = np.ascontiguousarray(bq[c0:c1].reshape(NHL, 128).T)
        bk_c = np.ascontiguousarray(bk[c0:c1].reshape(NHL, 128).T)
        bu_c = np.ascontiguousarray(bu[c0:c1].reshape(NHL, 128).T)
        bvb_c = np.ascontiguousarray(
            np.broadcast_to(bv[c0:c1][None, :], (128, NHL * HD))).astype(npdt)
        # atab[h, r, y] = table[y - r (+off)].  For the causal variant only
        # the columns at and above the diagonal are referenced, so the
        # shifted table is sliced to [MAXLEN-1:] (1024 wide; d0 = qoff -
        # jb*128).  Masked entries are -1e5 -> silu gives exactly 0.
        y = np.arange(2047)[None, :]
        r = np.arange(128)[:, None]
        idx = y - r
        valid = (idx >= 0) & (idx <= 2 * MAXLEN - 2)
        idxc = np.clip(idx, 0, 2 * MAXLEN - 2)
        cols = rel_table[:, g * NHL:(g + 1) * NHL]   # [2047, NHL]
        if causal:
            cols = np.where(np.arange(2047)[:, None] >= MAXLEN - 1, cols,
                            np.float32(-1e5))
            at = np.where(valid[:, :, None], cols[idxc], np.float32(-1e5))
            at = at[:, MAXLEN - 1:, :]            # [128, 1024, NHL]
        else:
            at = cols[idxc] * valid[:, :, None]   # [128, 2047, NHL]
        atab_c = np.ascontiguousarray(at.transpose(2, 0, 1)).astype(npdt)
        gdata.append((wq_c, wk_c, wv_c, wu_c, wf2_c, bq_c, bk_c, bu_c,
                      bvb_c, atab_c))

    for c in range(NCORES):
        b, g = c // HGRP, c % HGRP
        (wq_c, wk_c, wv_c, wu_c, wf2_c, bq_c, bk_c, bu_c, bvb_c,
         atab_c) = gdata[g]
        m = {
            "qT": np.ascontiguousarray(query[b].T).astype(npdt),
            "kT": np.ascontiguousarray(key[b].T).astype(npdt),
            "vT": np.ascontiguousarray(value[b].T).astype(npdt),
            "wq": wq_c, "wk": wk_c, "wv": wv_c, "wu": wu_c, "wf2": wf2_c,
            "bq": bq_c, "bk": bk_c, "bu": bu_c, "bvb": bvb_c,
            "atab": atab_c,
        }
        if not causal:
            import ml_dtypes as _mld
            mb_ = attn_mask[b]
            mf = np.empty((128, NHL, S), _mld.bfloat16)
            for jb in range(8):
                mf[:, jb, :] = mb_[:, jb * 128:(jb + 1) * 128].T
            m["maskf"] = mf
        in_maps.append(m)
    return in_maps


def kernel(query, key, value, attn_mask, Wq, bq, Wk, bk, Wv, bv, Wu, bu,
           Wf2, bf2, rel_table):
    global LAST_EXEC_NS
    query = np.asarray(query, np.float32)
    key = np.asarray(key, np.float32)
    value = np.asarray(value, np.float32)
    attn_mask = np.asarray(attn_mask, bool)
    Wq, bq = np.asarray(Wq, np.float32), np.asarray(bq, np.float32)
    Wk, bk = np.asarray(Wk, np.float32), np.asarray(bk, np.float32)
    Wv, bv = np.asarray(Wv, np.float32), np.asarray(bv, np.float32)
    Wu, bu = np.asarray(Wu, np.float32), np.asarray(bu, np.float32)
    Wf2, bf2 = np.asarray(Wf2, np.float32), np.asarray(bf2, np.float32)
    rel_table = np.asarray(rel_table, np.float32)

    tril = np.tril(np.ones((S, S), bool))
    causal = all(np.array_equal(attn_mask[b], tril) for b in range(B))

    if causal not in _CACHE:
        _CACHE[causal] = _build(causal)
    nc = _CACHE[causal]

    in_maps = _host_shards(query, key, value, attn_mask, Wq, bq, Wk, bk,
                           Wv, bv, Wu, bu, Wf2, rel_table, causal)
    res = run_bass_kernel_spmd(nc, in_maps, list(range(NCORES)), trace=TRACE)
    if res.exec_time_ns is not None:
        LAST_EXEC_NS = res.exec_time_ns

    outp = np.empty((B, S, H), np.float32)
    for b in range(B):
        outp[b] = (res.results[2 * b]["out"].astype(np.float32)
                   + res.results[2 * b + 1]["out"].astype(np.float32)
                   + bf2[None, :])
    return outp


# revision 15
# speedup vs baseline: 1.0505x; 1.0505x over previous
"""Trainium2 Bass kernel for nn_BaselineModel_35175782154746 (dense transformer
block with SiLU attention + relative-position bias).

Sharding: 8 NeuronCores = 4 batches x 2 head-groups (8 heads each).
Each core computes, for its (batch b, head-group g):
    U, Q, K, V projections (columns g*1024:(g+1)*1024 of Wu/Wq/Wk/Wv),
    SiLU attention with rel-pos bias for its 8 heads,
    gated = out * U (written in place over U), partial = gated @ Wf2 rows.
Host reduces: out[b] = partial[2b] + partial[2b+1] + bf2.

v2 schedule (vs the 362.7us baseline):
  - causal fine-trim: score/AV blocks narrowed to the causal region at
    128-granularity; two trimmed blocks pack into one PSUM bank (the
    second with start=False, landing on the bank's pending-zero range).
  - projection halves end with the last 4 k-tiles h-grouped so PSUM
    banks free progressively instead of all at once behind a serial
    silu drain.
  - scores(ih=0) run between K and V, interleaved with V's sb-blocks;
    AV(ih=0) interleaves with V's second column half.  V bias is a DVE
    add (no ones-matmul).  gated overwrites UT in place (saves 2MB).
  - ih=1 loop is software-pipelined (AV of head h-1 after scores of h),
    f2 part-A blocks spread through it, f2 copies on the Scalar engine,
    per-(head,ih) merged silu, output staged/stored as bf16.
  - DMA issue spread over sync/scalar/vector/gpsimd queues.
"""

import sys
import os

for _p in ("/root/.axon_site/_ro/trn_rl_repo", "/opt/trn_rl_repo"):
    if os.path.isdir(_p) and _p not in sys.path:
        sys.path.append(_p)

import numpy as np

import concourse.bass as bass
import concourse.mybir as mybir
import concourse.tile as tile
from concourse import bacc
from concourse.bass_utils import run_bass_kernel_spmd

B, S, H, NH, MAXLEN = 4, 1024, 2048, 16, 1024
HD = H // NH            # 128
NHL = 8                 # heads per core (local)
HGRP = 2                # head groups
NCORES = 8
KT16 = H // 128         # 16 k-tiles for the H contraction
SCALE = float(HD) ** -0.5

f32 = mybir.dt.float32
bf16 = mybir.dt.bfloat16
SILU = mybir.ActivationFunctionType.Silu
MULT = mybir.AluOpType.mult
ADD = mybir.AluOpType.add

TRACE = False
LAST_EXEC_NS = None
_CACHE = {}


def _blocks(causal, ih):
    """Score blocks for query half ih, in PSUM-group order.

    Each entry is (jb, qoff, width, packed): qoff the absolute query
    offset, width the trimmed moving size; packed means the block shares
    the previous block's PSUM bank, written at the next free column.
    """
    if not causal:
        return [(jb, ih * 512, 512, False) for jb in range(8)]
    if ih == 0:
        # widths 512,384,256,128 -> bank groups [jb0], [jb1,jb3], [jb2]
        return [(0, 0, 512, False), (1, 128, 384, False), (3, 384, 128, True),
                (2, 256, 256, False)]
    # ih == 1: jb0..4 full, then [jb5,jb7] packed, jb6 alone
    out = [(jb, 512, 512, False) for jb in range(5)]
    out += [(5, 640, 384, False), (7, 896, 128, True), (6, 768, 256, False)]
    return out


def _att_layout(causal, ih):
    """att tile column ranges: dict jb -> (col_start, width, qoff)."""
    lay = {}
    c = 0
    for jb, qoff, w, packed in _blocks(causal, ih):
        lay[jb] = (c, w, qoff)
        c += w
    return lay, c


def _build(causal: bool):
    nc = bacc.Bacc("TRN2", target_bir_lowering=False, debug=False,
                   num_devices=NCORES)

    def din(name, shape, dt=f32):
        return nc.dram_tensor(name, shape, dt, kind="ExternalInput").ap()

    qT = din("qT", [H, S], bf16)
    kT = din("kT", [H, S], bf16)
    vT = din("vT", [H, S], bf16)
    wq = din("wq", [H, NHL * HD], bf16)
    wk = din("wk", [H, NHL * HD], bf16)
    wv = din("wv", [H, NHL * HD], bf16)
    wu = din("wu", [H, NHL * HD], bf16)
    wf2 = din("wf2", [NHL * HD, H], bf16)
    bq = din("bq", [128, NHL])
    bk = din("bk", [128, NHL])
    bu = din("bu", [128, NHL])
    bvb = din("bvb", [128, NHL * HD], bf16)
    ATW = 1024 if causal else 2047
    atab = din("atab", [NHL, 128, ATW], bf16)
    if not causal:
        maskf = din("maskf", [128, NHL, S], bf16)
    out = nc.dram_tensor("out", [S, H], bf16, kind="ExternalOutput").ap()

    wf2r = wf2.rearrange("(cb p) n -> p cb n", p=128)

    lay0, W0 = _att_layout(causal, 0)
    lay1, W1 = _att_layout(causal, 1)

    with tile.TileContext(nc) as tc:
        with (
            tc.tile_pool(name="const", bufs=1) as constp,
            tc.tile_pool(name="projout", bufs=1) as projp,
        ):
            bq_t = constp.tile([128, NHL], f32, tag="bq")
            bk_t = constp.tile([128, NHL], f32, tag="bk")
            bu_t = constp.tile([128, NHL], f32, tag="bu")
            bvb_t = constp.tile([128, NHL * HD], bf16, tag="bvb")

            UT = projp.tile([128, NHL, S], bf16, tag="UT")
            QT = projp.tile([128, NHL, S], bf16, tag="QT")
            KTt = projp.tile([128, NHL, S], bf16, tag="KT")
            V = projp.tile([128, NHL, S], bf16, tag="V")
            at_tiles = [projp.tile([128, ATW], bf16, tag=f"atab{h}",
                                   name=f"atab{h}")
                        for h in range(NHL)]
            if not causal:
                mask_t = projp.tile([128, NHL, S], bf16, tag="mask")

            # ---- emit helpers (pool passed per phase) ----
            def emit_scores(h, ih, att_t, lay, pool, tagger, silu_after=None):
                """score matmuls + STT(scale, +rel-bias) for (head, half).

                silu_after: list of (stt_idx, col_end) - emit a silu over
                att[prev_end:col_end] right after that STT.  None -> no
                silu here (caller emits it).
                """
                blocks = _blocks(causal, ih)
                at = at_tiles[h]
                scp = None
                gi = 0
                off = 0
                stts = []
                for bi, (jb, qoff, w, packed) in enumerate(blocks):
                    if not packed:
                        scp = pool.tile([128, 512], f32, tag=tagger(gi),
                                        name=f"sc{h}_{ih}_{jb}")
                        gi += 1
                        off = 0
                    grp_end = (bi + 1 >= len(blocks)
                               or not blocks[bi + 1][3])
                    nc.tensor.matmul(
                        scp[:, off:off + w],
                        lhsT=KTt[:, h, jb * 128:(jb + 1) * 128],
                        rhs=QT[:, h, qoff:qoff + w],
                        start=(not packed), stop=grp_end)
                    stts.append((scp, off, jb, qoff, w))
                    off += w
                silu_map = dict(silu_after or [])
                prev_end = 0
                for si, (scp, off, jb, qoff, w) in enumerate(stts):
                    c0 = lay[jb][0]
                    d0 = qoff - jb * 128 + (0 if causal else MAXLEN - 1)
                    nc.vector.scalar_tensor_tensor(
                        att_t[:, c0:c0 + w], scp[:, off:off + w],
                        SCALE, at[:, d0:d0 + w], op0=MULT, op1=ADD)
                    if si in silu_map:
                        ce = silu_map[si]
                        nc.scalar.activation(att_t[:, prev_end:ce],
                                             att_t[:, prev_end:ce], SILU)
                        prev_end = ce

            def emit_post(h, att_t, lay):
                """merged silu (+ mask for the dense variant)."""
                nc.scalar.activation(att_t[:], att_t[:], SILU)
                if not causal:
                    for jb in sorted(lay):
                        c0, w, qoff = lay[jb]
                        nc.vector.tensor_mul(
                            att_t[:, c0:c0 + w], att_t[:, c0:c0 + w],
                            mask_t[:, jb, qoff:qoff + w])

            def emit_av(h, ih, att_t, lay, avp):
                """AV accumulation + gated-mul written in place over UT."""
                base = ih * 512
                njb = len(lay)
                done = 0
                for jb in sorted(lay):
                    c0, w, qoff = lay[jb]
                    o = qoff - base
                    done += 1
                    nc.tensor.matmul(
                        avp[:, o:o + w],
                        lhsT=V[:, jb, h * HD:(h + 1) * HD],
                        rhs=att_t[:, c0:c0 + w],
                        start=(jb == 0), stop=(done == njb))
                nc.vector.tensor_mul(
                    UT[:, h, base:base + 512], avp[:],
                    UT[:, h, base:base + 512])

            with tc.tile_pool(name="inres", bufs=1) as inres:
                qres = inres.tile([128, KT16, S], bf16, tag="qres")
                kres = inres.tile([128, KT16, S], bf16, tag="kres")
                vres = inres.tile([128, KT16, S], bf16, tag="qres",
                                  name="vres")
                wvres = inres.tile([128, KT16, NHL * HD], bf16, tag="wv",
                                   name="wvres")
                # input DMAs, serialized on the sync queue so they never
                # starve the gpsimd weight stream: qres (k0 split for a
                # faster first matmul), kres, then vres (waits for Q to
                # release the qres slot, blocking the queue until ~Q-end),
                # atab, wvres (lands during K).  biases on scalar (tiny).
                nc.sync.dma_start(qres[:, 0, 0:512], qT[0:128, 0:512])
                nc.sync.dma_start(qres[:, 0, 512:S], qT[0:128, 512:S])
                for k in range(1, KT16):
                    nc.sync.dma_start(qres[:, k, :],
                                      qT[k * 128:(k + 1) * 128, :])
                nc.scalar.dma_start(bu_t[:], bu[:])
                nc.scalar.dma_start(bq_t[:], bq[:])
                nc.scalar.dma_start(bk_t[:], bk[:])
                nc.scalar.dma_start(bvb_t[:], bvb[:])
                for k in range(KT16):
                    nc.sync.dma_start(kres[:, k, :],
                                      kT[k * 128:(k + 1) * 128, :])
                for k in range(KT16):
                    nc.sync.dma_start(vres[:, k, :],
                                      vT[k * 128:(k + 1) * 128, :])
                for h in range(NHL):
                    nc.sync.dma_start(at_tiles[h][:], atab[h])
                if not causal:
                    nc.sync.dma_start(mask_t[:], maskf[:])
                for k in range(KT16):
                    nc.sync.dma_start(wvres[:, k, :],
                                      wv[k * 128:(k + 1) * 128, :])

                # ---------------- projections U, Q, K ----------------
                with (
                    tc.tile_pool(name="win", bufs=7) as winp,
                    tc.tile_pool(name="pps", bufs=1, space="PSUM") as ppsum,
                ):
                    KSPLIT = 12   # k-outer prefix; tail h-grouped

                    for wdram, xres, btile, outtile, st_i in (
                        (wu, qres, bu_t, UT, 0),
                        (wq, qres, bq_t, QT, 1),
                        (wk, kres, bk_t, KTt, 2),
                    ):
                        for ih in range(2):
                            wts = {}
                            for k in range(KT16):
                                wt = winp.tile([128, NHL * HD], bf16,
                                               tag="win")
                                if st_i == 0 and ih == 0 and k == 0:
                                    # split so the first matmul's slice
                                    # arrives as early as possible
                                    nc.gpsimd.dma_start(wt[:, 0:HD],
                                                        wdram[0:128, 0:HD])
                                    nc.gpsimd.dma_start(
                                        wt[:, HD:], wdram[0:128, HD:])
                                else:
                                    nc.gpsimd.dma_start(
                                        wt[:],
                                        wdram[k * 128:(k + 1) * 128, :])
                                wts[k] = wt
                            ps = [ppsum.tile([128, 512], f32, tag=f"pp{h}",
                                             name=f"pp{st_i}_{ih}_{h}")
                                  for h in range(NHL)]
                            sl = slice(ih * 512, (ih + 1) * 512)
                            for k in range(KSPLIT):
                                for h in range(NHL):
                                    nc.tensor.matmul(
                                        ps[h][:],
                                        lhsT=wts[k][:, h * HD:(h + 1) * HD],
                                        rhs=xres[:, k, sl],
                                        start=(k == 0), stop=False)
                            for h in range(NHL):
                                for k in range(KSPLIT, KT16):
                                    nc.tensor.matmul(
                                        ps[h][:],
                                        lhsT=wts[k][:, h * HD:(h + 1) * HD],
                                        rhs=xres[:, k, sl],
                                        start=False, stop=(k == KT16 - 1))
                                nc.scalar.activation(
                                    outtile[:, h, sl], ps[h][:], SILU,
                                    bias=btile[:, h:h + 1])

                # ------------- scores(ih=0) + V + AV(ih=0) -------------
                with (
                    tc.tile_pool(name="att0p", bufs=1) as att0p,
                    tc.tile_pool(name="psA", bufs=1, space="PSUM") as psA,
                ):
                    att0 = [att0p.tile([128, W0], bf16, tag=f"att0_{h}",
                                       name=f"att0_{h}")
                            for h in range(NHL)]

                    def a_tag(i):
                        return f"s{i % 3}_{(i // 3) % 2}"

                    def emit_v_block(ch, sb):
                        csl = slice(ch * 512, (ch + 1) * 512)
                        pv = psA.tile([128, 512], f32, tag=f"v{sb % 2}",
                                      name=f"v{ch}_{sb}")
                        for k in range(KT16):
                            nc.tensor.matmul(
                                pv[:],
                                lhsT=vres[:, k, sb * 128:(sb + 1) * 128],
                                rhs=wvres[:, k, csl],
                                start=(k == 0), stop=(k == KT16 - 1))
                        nc.vector.tensor_add(V[:, sb, csl], pv[:],
                                             bvb_t[:, csl])
                        nc.scalar.activation(V[:, sb, csl],
                                             V[:, sb, csl], SILU)

                    # loop1: scores-ih0 head i interleaved with V ch0
                    for i in range(NHL):
                        emit_scores(i, 0, att0[i], lay0, psA, a_tag)
                        emit_post(i, att0[i], lay0)
                        emit_v_block(0, i)
                    # loop2: V ch1 + AV-ih0 heads 0-3
                    for i in range(NHL):
                        emit_v_block(1, i)
                        if i % 2 == 1:
                            h = i // 2
                            avp = psA.tile([128, 512], f32,
                                           tag=f"s{h % 3}_0",
                                           name=f"av0_{h}")
                            emit_av(h, 0, att0[h], lay0, avp)
                    # loop3: AV-ih0 heads 4-7
                    for h in range(4, NHL):
                        avp = psA.tile([128, 512], f32, tag=f"s{h % 3}_1",
                                       name=f"av0_{h}")
                        emit_av(h, 0, att0[h], lay0, avp)

            # ---------------- ih=1 attention + f2 ----------------
            with (
                tc.tile_pool(name="att1p", bufs=2) as att1p,
                tc.tile_pool(name="psB", bufs=1, space="PSUM") as psB,
                tc.tile_pool(name="w2p", bufs=8) as w2p,
                tc.tile_pool(name="stgp", bufs=4) as stgp,
            ):
                w2a = []
                for n in range(4):
                    t = w2p.tile([128, NHL, 512], bf16, tag="w2",
                                 name=f"w2a{n}")
                    nc.scalar.dma_start(t[:],
                                        wf2r[:, :, n * 512:(n + 1) * 512])
                    w2a.append(t)

                def b_tag(i):
                    return f"A{i % 4}"

                F2TAGS = ("B0", "B1", "C")

                def emit_f2_block(w2t, n, sb, which):
                    ps = psB.tile([128, 512], f32, tag=F2TAGS[which % 3],
                                  name=f"f2_{n}_{sb}")
                    for cb in range(NHL):
                        nc.tensor.matmul(
                            ps[:],
                            lhsT=UT[:, cb, sb * 128:(sb + 1) * 128],
                            rhs=w2t[:, cb, :],
                            start=(cb == 0), stop=(cb == NHL - 1))
                    st = stgp.tile([128, 512], bf16, tag="st",
                                   name=f"st{n}_{sb}")
                    nc.scalar.copy(st[:], ps[:])
                    nc.sync.dma_start(
                        out[sb * 128:(sb + 1) * 128,
                            n * 512:(n + 1) * 512], st[:])

                fa = [(n, sb) for n in range(4) for sb in range(4)]
                f2_sched = [1, 2, 3, 2, 2, 2, 2, 2]
                # causal ih1 STTs: 5 full groups then jb5, jb7, jb6 -> silu
                # [0:2560] after stt#4 and [2560:W1] after stt#7
                silu1 = ([(4, 2560), (7, W1)] if causal else None)
                fpos = 0
                w2b = []
                att1_tiles = {}
                for i in range(NHL):
                    att1_tiles[i] = att1p.tile([128, W1], bf16, tag="att1",
                                               name=f"att1_{i}")
                    emit_scores(i, 1, att1_tiles[i], lay1, psB, b_tag,
                                silu_after=silu1)
                    for n, sb in fa[fpos:fpos + f2_sched[i]]:
                        emit_f2_block(w2a[n], n, sb, fpos)
                        fpos += 1
                    if not causal:
                        emit_post(i, att1_tiles[i], lay1)
                    if i > 0:
                        avp = psB.tile([128, 512], f32, tag="D",
                                       name=f"av1_{i - 1}")
                        emit_av(i - 1, 1, att1_tiles.pop(i - 1), lay1, avp)
                    if i % 2 == 1:
                        t = w2p.tile([128, NHL, 512], bf16, tag="w2",
                                     name=f"w2b{i // 2}")
                        nc.gpsimd.dma_start(
                            t[:], wf2r[:, :, (i // 2) * 512:
                                       (i // 2 + 1) * 512])
                        w2b.append(t)
                avp = psB.tile([128, 512], f32, tag="D", name="av1_7")
                emit_av(7, 1, att1_tiles.pop(7), lay1, avp)

                fb = [(n, sb) for n in range(4) for sb in range(4, 8)]
                for idx, (n, sb) in enumerate(fb):
                    emit_f2_block(w2b[n], n, sb, idx)

    nc.compile()
    return nc


def _host_shards(query, key, value, attn_mask, Wq, bq, Wk, bk, Wv, bv,
                 Wu, bu, Wf2, rel_table, causal):
    """Build the per-core input maps."""
    import ml_dtypes
    npdt = np.dtype(ml_dtypes.bfloat16)
    in_maps = []
    gdata = []
    for g in range(HGRP):
        c0, c1 = g * NHL * HD, (g + 1) * NHL * HD
        wq_c = np.ascontiguousarray(Wq[:, c0:c1]).astype(npdt)
        wk_c = np.ascontiguousarray(Wk[:, c0:c1]).astype(npdt)
        wv_c = np.ascontiguousarray(Wv[:, c0:c1]).astype(npdt)
        wu_c = np.ascontiguousarray(Wu[:, c0:c1]).astype(npdt)
        wf2_c = np.ascontiguousarray(Wf2[c0:c1, :]).astype(npdt)
        bq_c = np.ascontiguousarray(bq[c0:c1].reshape(NHL, 128).T)
        bk_c = np.ascontiguousarray(bk[c0:c1].reshape(NHL, 128).T)
        bu_c = np.ascontiguousarray(bu[c0:c1].reshape(NHL, 128).T)
        bvb_c = np.ascontiguousarray(
            np.broadcast_to(bv[c0:c1][None, :], (128, NHL * HD))).astype(npdt)
        # atab[h, r, y] = table[y - r (+off)].  For the causal variant only
        # the columns at and above the diagonal are referenced, so the
        # shifted table is sliced to [MAXLEN-1:] (1024 wide; d0 = qoff -
        # jb*128).  Masked entries are -1e5 -> silu gives exactly 0.
        y = np.arange(2047)[None, :]
        r = np.arange(128)[:, None]
        idx = y - r
        valid = (idx >= 0) & (idx <= 2 * MAXLEN - 2)
        idxc = np.clip(idx, 0, 2 * MAXLEN - 2)
        cols = rel_table[:, g * NHL:(g + 1) * NHL]   # [2047, NHL]
        if causal:
            cols = np.where(np.arange(2047)[:, None] >= MAXLEN - 1, cols,
                            np.float32(-1e5))
            at = np.where(valid[:, :, None], cols[idxc], np.float32(-1e5))
            at = at[:, MAXLEN - 1:, :]            # [128, 1024, NHL]
        else:
            at = cols[idxc] * valid[:, :, None]   # [128, 2047, NHL]
        atab_c = np.ascontiguousarray(at.transpose(2, 0, 1)).astype(npdt)
        gdata.append((wq_c, wk_c, wv_c, wu_c, wf2_c, bq_c, bk_c, bu_c,
                      bvb_c, atab_c))

    for c in range(NCORES):
        b, g = c // HGRP, c % HGRP
        (wq_c, wk_c, wv_c, wu_c, wf2_c, bq_c, bk_c, bu_c, bvb_c,
         atab_c) = gdata[g]
        m = {
            "qT": np.ascontiguousarray(query[b].T).astype(npdt),
            "kT": np.ascontiguousarray(key[b].T).astype(npdt),
            "vT": np.ascontiguousarray(value[b].T).astype(npdt),
            "wq": wq_c, "wk": wk_c, "wv": wv_c, "wu": wu_c, "wf2": wf2_c,
            "bq": bq_c, "bk": bk_c, "bu": bu_c, "bvb": bvb_c,
            "atab": atab_c,
        }
        if not causal:
            import ml_dtypes as _mld
            mb_ = attn_mask[b]
            mf = np.empty((128, NHL, S), _mld.bfloat16)
            for jb in range(8):
                mf[:, jb, :] = mb_[:, jb * 128:(jb + 1) * 128].T
            m["maskf"] = mf
        in_maps.append(m)
    return in_maps


def kernel(query, key, value, attn_mask, Wq, bq, Wk, bk, Wv, bv, Wu, bu,
           Wf2, bf2, rel_table):
    global LAST_EXEC_NS
    query = np.asarray(query, np.float32)
    key = np.asarray(key, np.float32)
    value = np.asarray(value, np.float32)
    attn_mask = np.asarray(attn_mask, bool)
    Wq, bq = np.asarray(Wq, np.float32), np.asarray(bq, np.float32)
    Wk, bk = np.asarray(Wk, np.float32), np.asarray(bk, np.float32)
    Wv, bv = np.asarray(Wv, np.float32), np.asarray(bv, np.float32)
    Wu, bu = np.asarray(Wu, np.float32), np.asarray(bu, np.float32)
    Wf2, bf2 = np.asarray(Wf2, np.float32), np.asarray(bf2, np.float32)
    rel_table = np.asarray(rel_table, np.float32)

    tril = np.tril(np.ones((S, S), bool))
    causal = all(np.array_equal(attn_mask[b], tril) for b in range(B))

    if causal not in _CACHE:
        _CACHE[causal] = _build(causal)
    nc = _CACHE[causal]

    in_maps = _host_shards(query, key, value, attn_mask, Wq, bq, Wk, bk,
                           Wv, bv, Wu, bu, Wf2, rel_table, causal)
    res = run_bass_kernel_spmd(nc, in_maps, list(range(NCORES)), trace=TRACE)
    if res.exec_time_ns is not None:
        LAST_EXEC_NS = res.exec_time_ns

    outp = np.empty((B, S, H), np.float32)
    for b in range(B):
        outp[b] = (res.results[2 * b]["out"].astype(np.float32)
                   + res.results[2 * b + 1]["out"].astype(np.float32)
                   + bf2[None, :])
    return outp


# revision 22
# speedup vs baseline: 1.0697x; 1.0183x over previous
"""Trainium2 Bass kernel for nn_BaselineModel_35175782154746 (dense transformer
block with SiLU attention + relative-position bias).

Sharding: 8 NeuronCores = 4 batches x 2 head-groups (8 heads each).
Each core computes, for its (batch b, head-group g):
    U, Q, K, V projections (columns g*1024:(g+1)*1024 of Wu/Wq/Wk/Wv),
    SiLU attention with rel-pos bias for its 8 heads,
    gated = out * U (written in place over U), partial = gated @ Wf2 rows.
Host reduces: out[b] = partial[2b] + partial[2b+1] + bf2.

v2 schedule (vs the 362.7us baseline):
  - causal fine-trim: score/AV blocks narrowed to the causal region at
    128-granularity; two trimmed blocks pack into one PSUM bank (the
    second with start=False, landing on the bank's pending-zero range).
  - projection halves end with the last 4 k-tiles h-grouped so PSUM
    banks free progressively instead of all at once behind a serial
    silu drain.
  - scores(ih=0) run between K and V, interleaved with V's sb-blocks;
    AV(ih=0) interleaves with V's second column half.  V bias is a DVE
    add (no ones-matmul).  gated overwrites UT in place (saves 2MB).
  - ih=1 loop is software-pipelined (AV of head h-1 after scores of h),
    f2 part-A blocks spread through it, f2 copies on the Scalar engine,
    per-(head,ih) merged silu, output staged/stored as bf16.
  - DMA issue spread over sync/scalar/vector/gpsimd queues.
"""

import sys
import os

for _p in ("/root/.axon_site/_ro/trn_rl_repo", "/opt/trn_rl_repo"):
    if os.path.isdir(_p) and _p not in sys.path:
        sys.path.append(_p)

import numpy as np

import concourse.bass as bass
import concourse.mybir as mybir
import concourse.tile as tile
from concourse import bacc
from concourse.bass_utils import run_bass_kernel_spmd

B, S, H, NH, MAXLEN = 4, 1024, 2048, 16, 1024
HD = H // NH            # 128
NHL = 8                 # heads per core (local)
HGRP = 2                # head groups
NCORES = 8
KT16 = H // 128         # 16 k-tiles for the H contraction
SCALE = float(HD) ** -0.5

f32 = mybir.dt.float32
bf16 = mybir.dt.bfloat16
SILU = mybir.ActivationFunctionType.Silu
MULT = mybir.AluOpType.mult
ADD = mybir.AluOpType.add

TRACE = False
LAST_EXEC_NS = None
_CACHE = {}


def _blocks(causal, ih):
    """Score blocks for query half ih, in PSUM-group order.

    Each entry is (jb, qoff, width, packed): qoff the absolute query
    offset, width the trimmed moving size; packed means the block shares
    the previous block's PSUM bank, written at the next free column.
    """
    if not causal:
        return [(jb, ih * 512, 512, False) for jb in range(8)]
    if ih == 0:
        # widths 512,384,256,128 -> bank groups [jb0], [jb1,jb3], [jb2]
        return [(0, 0, 512, False), (1, 128, 384, False), (3, 384, 128, True),
                (2, 256, 256, False)]
    # ih == 1: jb0..4 full, then [jb5,jb7] packed, jb6 alone
    out = [(jb, 512, 512, False) for jb in range(5)]
    out += [(5, 640, 384, False), (7, 896, 128, True), (6, 768, 256, False)]
    return out


def _att_layout(causal, ih):
    """att tile column ranges: dict jb -> (col_start, width, qoff)."""
    lay = {}
    c = 0
    for jb, qoff, w, packed in _blocks(causal, ih):
        lay[jb] = (c, w, qoff)
        c += w
    return lay, c


def _build(causal: bool):
    nc = bacc.Bacc("TRN2", target_bir_lowering=False, debug=False,
                   num_devices=NCORES)

    def din(name, shape, dt=f32):
        return nc.dram_tensor(name, shape, dt, kind="ExternalInput").ap()

    qT = din("qT", [H, S], bf16)
    kT = din("kT", [H, S], bf16)
    vT = din("vT", [H, S], bf16)
    wq = din("wq", [H, NHL * HD], bf16)
    wk = din("wk", [H, NHL * HD], bf16)
    wv = din("wv", [H, NHL * HD], bf16)
    wu = din("wu", [H, NHL * HD], bf16)
    wf2s = din("wf2s", [4, 128, NHL, 512], bf16)
    bq = din("bq", [128, NHL])
    bk = din("bk", [128, NHL])
    bu = din("bu", [128, NHL])
    bvb = din("bvb", [128, NHL * HD], bf16)
    ATW = 1024 if causal else 2047
    atab = din("atab", [NHL, 128, ATW], bf16)
    if not causal:
        maskf = din("maskf", [128, NHL, S], bf16)
    out = nc.dram_tensor("out", [S, H], bf16, kind="ExternalOutput").ap()

    lay0, W0 = _att_layout(causal, 0)
    lay1, W1 = _att_layout(causal, 1)

    with tile.TileContext(nc) as tc:
        with (
            tc.tile_pool(name="const", bufs=1) as constp,
            tc.tile_pool(name="projout", bufs=1) as projp,
        ):
            bq_t = constp.tile([128, NHL], f32, tag="bq")
            bk_t = constp.tile([128, NHL], f32, tag="bk")
            bu_t = constp.tile([128, NHL], f32, tag="bu")
            bvb_t = constp.tile([128, NHL * HD], bf16, tag="bvb")

            UT = projp.tile([128, NHL, S], bf16, tag="UT")
            QT = projp.tile([128, NHL, S], bf16, tag="QT")
            KTt = projp.tile([128, NHL, S], bf16, tag="KT")
            V = projp.tile([128, NHL, S], bf16, tag="V")
            at_tiles = [projp.tile([128, ATW], bf16, tag=f"atab{h}",
                                   name=f"atab{h}")
                        for h in range(NHL)]
            if not causal:
                mask_t = projp.tile([128, NHL, S], bf16, tag="mask")

            # ---- emit helpers (pool passed per phase) ----
            def emit_scores(h, ih, att_t, lay, pool, tagger, silu_after=None):
                """score matmuls + STT(scale, +rel-bias) for (head, half).

                silu_after: list of (stt_idx, col_end) - emit a silu over
                att[prev_end:col_end] right after that STT.  None -> no
                silu here (caller emits it).
                """
                blocks = _blocks(causal, ih)
                at = at_tiles[h]
                scp = None
                gi = 0
                off = 0
                stts = []
                for bi, (jb, qoff, w, packed) in enumerate(blocks):
                    if not packed:
                        scp = pool.tile([128, 512], f32, tag=tagger(gi),
                                        name=f"sc{h}_{ih}_{jb}")
                        gi += 1
                        off = 0
                    grp_end = (bi + 1 >= len(blocks)
                               or not blocks[bi + 1][3])
                    nc.tensor.matmul(
                        scp[:, off:off + w],
                        lhsT=KTt[:, h, jb * 128:(jb + 1) * 128],
                        rhs=QT[:, h, qoff:qoff + w],
                        start=(not packed), stop=grp_end)
                    stts.append((scp, off, jb, qoff, w))
                    off += w
                silu_map = dict(silu_after or [])
                prev_end = 0
                for si, (scp, off, jb, qoff, w) in enumerate(stts):
                    c0 = lay[jb][0]
                    d0 = qoff - jb * 128 + (0 if causal else MAXLEN - 1)
                    nc.vector.scalar_tensor_tensor(
                        att_t[:, c0:c0 + w], scp[:, off:off + w],
                        SCALE, at[:, d0:d0 + w], op0=MULT, op1=ADD)
                    if si in silu_map:
                        ce = silu_map[si]
                        nc.scalar.activation(att_t[:, prev_end:ce],
                                             att_t[:, prev_end:ce], SILU)
                        prev_end = ce

            def emit_post(h, att_t, lay):
                """merged silu (+ mask for the dense variant)."""
                nc.scalar.activation(att_t[:], att_t[:], SILU)
                if not causal:
                    for jb in sorted(lay):
                        c0, w, qoff = lay[jb]
                        nc.vector.tensor_mul(
                            att_t[:, c0:c0 + w], att_t[:, c0:c0 + w],
                            mask_t[:, jb, qoff:qoff + w])

            def emit_av(h, ih, att_t, lay, avp):
                """AV accumulation + gated-mul written in place over UT."""
                base = ih * 512
                njb = len(lay)
                done = 0
                for jb in sorted(lay):
                    c0, w, qoff = lay[jb]
                    o = qoff - base
                    done += 1
                    nc.tensor.matmul(
                        avp[:, o:o + w],
                        lhsT=V[:, jb, h * HD:(h + 1) * HD],
                        rhs=att_t[:, c0:c0 + w],
                        start=(jb == 0), stop=(done == njb))
                nc.vector.tensor_mul(
                    UT[:, h, base:base + 512], avp[:],
                    UT[:, h, base:base + 512])

            with tc.tile_pool(name="inres", bufs=1) as inres:
                qres = inres.tile([128, KT16, S], bf16, tag="qres")
                kres = inres.tile([128, KT16, S], bf16, tag="kres")
                vres = inres.tile([128, KT16, S], bf16, tag="qres",
                                  name="vres")
                wvres = inres.tile([128, KT16, NHL * HD], bf16, tag="wv",
                                   name="wvres")
                # input DMAs, serialized on the sync queue so they never
                # starve the gpsimd weight stream: qres (k0 split for a
                # faster first matmul), kres, then vres (waits for Q to
                # release the qres slot, blocking the queue until ~Q-end),
                # atab, wvres (lands during K).  biases on scalar (tiny).
                nc.sync.dma_start(qres[:, 0, 0:512], qT[0:128, 0:512])
                nc.sync.dma_start(qres[:, 0, 512:S], qT[0:128, 512:S])
                for k in range(1, KT16):
                    nc.sync.dma_start(qres[:, k, :],
                                      qT[k * 128:(k + 1) * 128, :])
                nc.scalar.dma_start(bu_t[:], bu[:])
                nc.scalar.dma_start(bq_t[:], bq[:])
                nc.scalar.dma_start(bk_t[:], bk[:])
                nc.scalar.dma_start(bvb_t[:], bvb[:])
                for k in range(KT16):
                    nc.sync.dma_start(kres[:, k, :],
                                      kT[k * 128:(k + 1) * 128, :])
                for k in range(KT16):
                    nc.sync.dma_start(vres[:, k, :],
                                      vT[k * 128:(k + 1) * 128, :])
                for h in range(NHL):
                    nc.sync.dma_start(at_tiles[h][:], atab[h])
                if not causal:
                    nc.sync.dma_start(mask_t[:], maskf[:])
                for k in range(KT16):
                    nc.sync.dma_start(wvres[:, k, :],
                                      wv[k * 128:(k + 1) * 128, :])

                # ---------------- projections U, Q, K ----------------
                with (
                    tc.tile_pool(name="win", bufs=12) as winp,
                    tc.tile_pool(name="pps", bufs=1, space="PSUM") as ppsum,
                ):
                    KSPLIT = 12   # k-outer prefix; tail h-grouped

                    for wdram, xres, btile, outtile, st_i in (
                        (wu, qres, bu_t, UT, 0),
                        (wq, qres, bq_t, QT, 1),
                        (wk, kres, bk_t, KTt, 2),
                    ):
                        for ih in range(2):
                            wts = {}
                            for k in range(KT16):
                                wt = winp.tile([128, NHL * HD], bf16,
                                               tag="win")
                                if st_i == 0 and ih == 0 and k == 0:
                                    # split so the first matmul's slice
                                    # arrives as early as possible
                                    nc.gpsimd.dma_start(wt[:, 0:HD],
                                                        wdram[0:128, 0:HD])
                                    nc.gpsimd.dma_start(
                                        wt[:, HD:], wdram[0:128, HD:])
                                else:
                                    nc.gpsimd.dma_start(
                                        wt[:],
                                        wdram[k * 128:(k + 1) * 128, :])
                                wts[k] = wt
                            ps = [ppsum.tile([128, 512], f32, tag=f"pp{h}",
                                             name=f"pp{st_i}_{ih}_{h}")
                                  for h in range(NHL)]
                            sl = slice(ih * 512, (ih + 1) * 512)
                            for k in range(KSPLIT):
                                for h in range(NHL):
                                    nc.tensor.matmul(
                                        ps[h][:],
                                        lhsT=wts[k][:, h * HD:(h + 1) * HD],
                                        rhs=xres[:, k, sl],
                                        start=(k == 0), stop=False)
                            for h in range(NHL):
                                for k in range(KSPLIT, KT16):
                                    nc.tensor.matmul(
                                        ps[h][:],
                                        lhsT=wts[k][:, h * HD:(h + 1) * HD],
                                        rhs=xres[:, k, sl],
                                        start=False, stop=(k == KT16 - 1))
                                nc.scalar.activation(
                                    outtile[:, h, sl], ps[h][:], SILU,
                                    bias=btile[:, h:h + 1])

                # f2 weights into the slot kres just vacated (tag reuse
                # delays the transfers to K-end; host layout is per-
                # partition contiguous).  One copy serves f2A and f2B.
                w2all = inres.tile([128, 4, NHL, 512], bf16, tag="kres",
                                   name="w2all")
                for n in range(4):
                    nc.scalar.dma_start(w2all[:, n], wf2s[n])

                # ------------- scores(ih=0) + V + AV(ih=0) -------------
                with (
                    tc.tile_pool(name="att0p", bufs=1) as att0p,
                    tc.tile_pool(name="psA", bufs=1, space="PSUM") as psA,
                ):
                    att0 = [att0p.tile([128, W0], bf16, tag=f"att0_{h}",
                                       name=f"att0_{h}")
                            for h in range(NHL)]

                    def a_tag(i):
                        return f"s{i % 3}_{(i // 3) % 2}"

                    def emit_v_block(ch, sb):
                        csl = slice(ch * 512, (ch + 1) * 512)
                        pv = psA.tile([128, 512], f32, tag=f"v{sb % 2}",
                                      name=f"v{ch}_{sb}")
                        for k in range(KT16):
                            nc.tensor.matmul(
                                pv[:],
                                lhsT=vres[:, k, sb * 128:(sb + 1) * 128],
                                rhs=wvres[:, k, csl],
                                start=(k == 0), stop=(k == KT16 - 1))
                        nc.vector.tensor_add(V[:, sb, csl], pv[:],
                                             bvb_t[:, csl])
                        nc.scalar.activation(V[:, sb, csl],
                                             V[:, sb, csl], SILU)

                    # loop1: scores-ih0 head i interleaved with V ch0
                    for i in range(NHL):
                        emit_scores(i, 0, att0[i], lay0, psA, a_tag)
                        emit_post(i, att0[i], lay0)
                        emit_v_block(0, i)
                    # loop2: V ch1 + AV-ih0 heads 0-3
                    for i in range(NHL):
                        emit_v_block(1, i)
                        if i % 2 == 1:
                            h = i // 2
                            avp = psA.tile([128, 512], f32,
                                           tag=f"s{h % 3}_0",
                                           name=f"av0_{h}")
                            emit_av(h, 0, att0[h], lay0, avp)
                    # loop3: AV-ih0 heads 4-7
                    for h in range(4, NHL):
                        avp = psA.tile([128, 512], f32, tag=f"s{h % 3}_1",
                                       name=f"av0_{h}")
                        emit_av(h, 0, att0[h], lay0, avp)

                # ---------------- ih=1 attention + f2 ----------------
                with (
                    tc.tile_pool(name="att1p", bufs=2) as att1p,
                    tc.tile_pool(name="psB", bufs=1, space="PSUM") as psB,
                    tc.tile_pool(name="stgp", bufs=4) as stgp,
                ):
                    def b_tag(i):
                        return f"A{i % 4}"

                    F2TAGS = ("B0", "B1", "C")

                    def emit_f2_block(n, sb, which):
                        ps = psB.tile([128, 512], f32,
                                      tag=F2TAGS[which % 3],
                                      name=f"f2_{n}_{sb}")
                        for cb in range(NHL):
                            nc.tensor.matmul(
                                ps[:],
                                lhsT=UT[:, cb, sb * 128:(sb + 1) * 128],
                                rhs=w2all[:, n, cb, :],
                                start=(cb == 0), stop=(cb == NHL - 1))
                        st = stgp.tile([128, 512], bf16, tag="st",
                                       name=f"st{n}_{sb}")
                        nc.scalar.copy(st[:], ps[:])
                        nc.sync.dma_start(
                            out[sb * 128:(sb + 1) * 128,
                                n * 512:(n + 1) * 512], st[:])

                    fa = [(n, sb) for n in range(4) for sb in range(4)]
                    f2_sched = [2, 2, 2, 2, 2, 2, 2, 2]
                    # causal ih1 STT order: 5 full groups then jb5, jb7,
                    # jb6 -> silu [0:2560] after stt#4, [2560:W1] after #7
                    silu1 = ([(4, 2560), (7, W1)] if causal else None)
                    fpos = 0
                    att1_tiles = {}
                    for i in range(NHL):
                        att1_tiles[i] = att1p.tile([128, W1], bf16,
                                                   tag="att1",
                                                   name=f"att1_{i}")
                        emit_scores(i, 1, att1_tiles[i], lay1, psB, b_tag,
                                    silu_after=silu1)
                        for n, sb in fa[fpos:fpos + f2_sched[i]]:
                            emit_f2_block(n, sb, fpos)
                            fpos += 1
                        if not causal:
                            emit_post(i, att1_tiles[i], lay1)
                        if i > 0:
                            avp = psB.tile([128, 512], f32, tag="D",
                                           name=f"av1_{i - 1}")
                            emit_av(i - 1, 1, att1_tiles.pop(i - 1), lay1,
                                    avp)
                    avp = psB.tile([128, 512], f32, tag="D", name="av1_7")
                    emit_av(7, 1, att1_tiles.pop(7), lay1, avp)

                    fb = [(n, sb) for n in range(4) for sb in range(4, 8)]
                    for idx, (n, sb) in enumerate(fb):
                        emit_f2_block(n, sb, idx)

    nc.compile()
    return nc


def _host_shards(query, key, value, attn_mask, Wq, bq, Wk, bk, Wv, bv,
                 Wu, bu, Wf2, rel_table, causal):
    """Build the per-core input maps."""
    import ml_dtypes
    npdt = np.dtype(ml_dtypes.bfloat16)
    in_maps = []
    gdata = []
    for g in range(HGRP):
        c0, c1 = g * NHL * HD, (g + 1) * NHL * HD
        wq_c = np.ascontiguousarray(Wq[:, c0:c1]).astype(npdt)
        wk_c = np.ascontiguousarray(Wk[:, c0:c1]).astype(npdt)
        wv_c = np.ascontiguousarray(Wv[:, c0:c1]).astype(npdt)
        wu_c = np.ascontiguousarray(Wu[:, c0:c1]).astype(npdt)
        # wf2s[n, p, cb, :] = Wf2[c0 + cb*128 + p, n*512:(n+1)*512] --
        # per-partition contiguous so the DMA streams cleanly
        wf2_c = np.ascontiguousarray(
            Wf2[c0:c1, :].reshape(NHL, 128, 4, 512)
            .transpose(2, 1, 0, 3)).astype(npdt)
        bq_c = np.ascontiguousarray(bq[c0:c1].reshape(NHL, 128).T)
        bk_c = np.ascontiguousarray(bk[c0:c1].reshape(NHL, 128).T)
        bu_c = np.ascontiguousarray(bu[c0:c1].reshape(NHL, 128).T)
        bvb_c = np.ascontiguousarray(
            np.broadcast_to(bv[c0:c1][None, :], (128, NHL * HD))).astype(npdt)
        # atab[h, r, y] = table[y - r (+off)].  For the causal variant only
        # the columns at and above the diagonal are referenced, so the
        # shifted table is sliced to [MAXLEN-1:] (1024 wide; d0 = qoff -
        # jb*128).  Masked entries are -1e5 -> silu gives exactly 0.
        y = np.arange(2047)[None, :]
        r = np.arange(128)[:, None]
        idx = y - r
        valid = (idx >= 0) & (idx <= 2 * MAXLEN - 2)
        idxc = np.clip(idx, 0, 2 * MAXLEN - 2)
        cols = rel_table[:, g * NHL:(g + 1) * NHL]   # [2047, NHL]
        if causal:
            cols = np.where(np.arange(2047)[:, None] >= MAXLEN - 1, cols,
                            np.float32(-1e5))
            at = np.where(valid[:, :, None], cols[idxc], np.float32(-1e5))
            at = at[:, MAXLEN - 1:, :]            # [128, 1024, NHL]
        else:
            at = cols[idxc] * valid[:, :, None]   # [128, 2047, NHL]
        atab_c = np.ascontiguousarray(at.transpose(2, 0, 1)).astype(npdt)
        gdata.append((wq_c, wk_c, wv_c, wu_c, wf2_c, bq_c, bk_c, bu_c,
                      bvb_c, atab_c))

    for c in range(NCORES):
        b, g = c // HGRP, c % HGRP
        (wq_c, wk_c, wv_c, wu_c, wf2_c, bq_c, bk_c, bu_c, bvb_c,
         atab_c) = gdata[g]
        m = {
            "qT": np.ascontiguousarray(query[b].T).astype(npdt),
            "kT": np.ascontiguousarray(key[b].T).astype(npdt),
            "vT": np.ascontiguousarray(value[b].T).astype(npdt),
            "wq": wq_c, "wk": wk_c, "wv": wv_c, "wu": wu_c, "wf2s": wf2_c,
            "bq": bq_c, "bk": bk_c, "bu": bu_c, "bvb": bvb_c,
            "atab": atab_c,
        }
        if not causal:
            import ml_dtypes as _mld
            mb_ = attn_mask[b]
            mf = np.empty((128, NHL, S), _mld.bfloat16)
            for jb in range(8):
                mf[:, jb, :] = mb_[:, jb * 128:(jb + 1) * 128].T
            m["maskf"] = mf
        in_maps.append(m)
    return in_maps


def kernel(query, key, value, attn_mask, Wq, bq, Wk, bk, Wv, bv, Wu, bu,
           Wf2, bf2, rel_table):
    global LAST_EXEC_NS
    query = np.asarray(query, np.float32)
    key = np.asarray(key, np.float32)
    value = np.asarray(value, np.float32)
    attn_mask = np.asarray(attn_mask, bool)
    Wq, bq = np.asarray(Wq, np.float32), np.asarray(bq, np.float32)
    Wk, bk = np.asarray(Wk, np.float32), np.asarray(bk, np.float32)
    Wv, bv = np.asarray(Wv, np.float32), np.asarray(bv, np.float32)
    Wu, bu = np.asarray(Wu, np.float32), np.asarray(bu, np.float32)
    Wf2, bf2 = np.asarray(Wf2, np.float32), np.asarray(bf2, np.float32)
    rel_table = np.asarray(rel_table, np.float32)

    tril = np.tril(np.ones((S, S), bool))
    causal = all(np.array_equal(attn_mask[b], tril) for b in range(B))

    if causal not in _CACHE:
        _CACHE[causal] = _build(causal)
    nc = _CACHE[causal]

    in_maps = _host_shards(query, key, value, attn_mask, Wq, bq, Wk, bk,
                           Wv, bv, Wu, bu, Wf2, rel_table, causal)
    res = run_bass_kernel_spmd(nc, in_maps, list(range(NCORES)), trace=TRACE)
    if res.exec_time_ns is not None:
        LAST_EXEC_NS = res.exec_time_ns

    outp = np.empty((B, S, H), np.float32)
    for b in range(B):
        outp[b] = (res.results[2 * b]["out"].astype(np.float32)
                   + res.results[2 * b + 1]["out"].astype(np.float32)
                   + bf2[None, :])
    return outp
